# revision 1
# baseline (speedup 1.0000x reference)
"""CIN (xDeepFM Compressed Interaction Network) Bass/Tile kernel for TRN2.

Problem: X_0 [1024, 39, 64]; three CIN layers (units 128 each):
    had_i = outer(X_0, X_i) over channel dims, per (b, d)
    X_{i+1} = W_i @ had_i + b_i            (1x1 conv over channels)
    pooled_i = X_{i+1}.sum(d)
Output: concat(pooled_1..3) -> [1024, 384] fp32.

Strategy (pure data-parallel over batch, 8 cores, 128 samples each):
  * channel-major layout on chip: tensors stored [channels, b*64+d]
  * had formation: DVE tensor_mul against broadcast tiles of X_0 rows
    (partition-broadcast DMA straight from compact DRAM, 0-stride APs)
  * convs: PE matmuls accumulating K-chunks in PSUM (K=117 for layer 1
    via 3-h groups; 39x128 chunks for layer 2)
  * evacuation: ScalarE Identity activation with per-partition bias and
    fused accum_out -> pooled sums come free
  * layer 3 never materializes its conv: pooled_3 = W3 @ Gram(X0, X2),
    with per-sample PE transposes + tiny Gram matmuls
  * elementwise dtype fp16 by default (DVE 2x mode); BASS_CIN_DTYPE=fp32
    falls back to full fp32.
"""

import os
import numpy as np

import concourse.bass as bass
import concourse.bacc as bacc
import concourse.mybir as mybir
import concourse.tile as tile
from concourse import bass_utils

F32 = mybir.dt.float32
F16 = mybir.dt.float16

B, F, D, U = 1024, 39, 64, 128
NCORES = 8
BC = B // NCORES            # 128 samples per core
BD = BC * D                 # 8192 bd-columns per core
T = 512                     # bd-tile width
NT = BD // T                # 16 tiles
S = T // D                  # 8 samples per tile
KG1 = 13                    # layer-1 chunks: 13 x (3 h-values x 39 m) = 117 rows

_CACHE: dict = {}


def _dtype_cfg():
    name = os.environ.get("BASS_CIN_DTYPE", "fp16")
    if name == "fp32":
        return F32, np.float32
    return F16, np.float16


def _build(dt_e) -> bacc.Bacc:
    nc = bacc.Bacc("TRN2", target_bir_lowering=False, debug=False,
                   enable_asserts=False)
    AF = mybir.ActivationFunctionType

    x0cp_d = nc.dram_tensor("x0cp", [F, BD], dt_e, kind="ExternalInput")
    x0dt_d = nc.dram_tensor("x0dt", [D, BC * F], dt_e, kind="ExternalInput")
    w1p_d = nc.dram_tensor("w1p", [117, KG1 * U], dt_e, kind="ExternalInput")
    w2p_d = nc.dram_tensor("w2p", [U, F * U], dt_e, kind="ExternalInput")
    w3p_d = nc.dram_tensor("w3p", [F, U * U], dt_e, kind="ExternalInput")
    b1_d = nc.dram_tensor("b1c", [U, 1], F32, kind="ExternalInput")
    b2_d = nc.dram_tensor("b2c", [U, 1], F32, kind="ExternalInput")
    b3_d = nc.dram_tensor("b3c", [U, 1], F32, kind="ExternalInput")  # 64*b3
    id16_d = nc.dram_tensor("id16", [U, U], dt_e, kind="ExternalInput")
    id32_d = nc.dram_tensor("id32", [U, U], F32, kind="ExternalInput")

    y_d = nc.dram_tensor("y", [BC, 3 * U], F32, kind="ExternalOutput")

    with tile.TileContext(nc) as tc:
        with (
            tc.tile_pool(name="static", bufs=1) as stat,
            tc.tile_pool(name="bc1", bufs=2) as bc1p,
            tc.tile_pool(name="bc2", bufs=2) as bc2p,
            tc.tile_pool(name="had", bufs=3) as hadp,
            tc.tile_pool(name="xsb", bufs=2) as xsbp,
            tc.tile_pool(name="l3sb", bufs=3) as l3p,
            tc.tile_pool(name="ps_x1", bufs=2, space="PSUM") as pp1,
            tc.tile_pool(name="ps_x2", bufs=2, space="PSUM") as pp2,
            tc.tile_pool(name="ps_tg", bufs=3, space="PSUM") as pptg,
        ):
            # ---- static loads ----
            x0st3 = stat.tile([117, BD], dt_e)          # X0 rows tiled 3x
            for j in range(3):
                nc.sync.dma_start(x0st3[j * F:(j + 1) * F, :], x0cp_d[:, :])
            w1sb = stat.tile([117, KG1 * U], dt_e)
            nc.sync.dma_start(w1sb[:], w1p_d[:, :])
            w2sb = stat.tile([U, F * U], dt_e)
            nc.sync.dma_start(w2sb[:], w2p_d[:, :])
            w3sb = stat.tile([F, U * U], dt_e)
            nc.sync.dma_start(w3sb[:], w3p_d[:, :])
            x0dt = stat.tile([D, BC * F], dt_e)
            nc.sync.dma_start(x0dt[:], x0dt_d[:, :])
            b1sb = stat.tile([U, 1], F32)
            nc.sync.dma_start(b1sb[:], b1_d[:, :])
            b2sb = stat.tile([U, 1], F32)
            nc.sync.dma_start(b2sb[:], b2_d[:, :])
            b3sb = stat.tile([U, 1], F32)
            nc.sync.dma_start(b3sb[:], b3_d[:, :])
            id16 = stat.tile([U, U], dt_e)
            nc.sync.dma_start(id16[:], id16_d[:, :])
            id32 = stat.tile([U, U], F32)
            nc.sync.dma_start(id32[:], id32_d[:, :])

            pooled1 = stat.tile([U, BC], F32)
            pooled2 = stat.tile([U, BC], F32)
            pooled3 = stat.tile([U, BC], F32)
            g2sb = stat.tile([F, BC, U], dt_e)          # Gram(X0, X2): [h, b, m]
            outsb = stat.tile([BC, 3 * U], F32)

            # ---- main loop over bd tiles ----
            for t in range(NT):
                cols = slice(t * T, (t + 1) * T)

                # broadcast slabs for this tile, straight from DRAM rows
                bc1 = bc1p.tile([117, KG1, T], dt_e)
                for j in range(3):
                    src = x0cp_d[j:F:3, cols].partition_broadcast(F)
                    nc.sync.dma_start(bc1[j * F:(j + 1) * F, :, :], src)
                bc2 = []
                for g in range(5):                       # groups of 8 h (last 7)
                    h0 = g * 8
                    cnt = min(8, F - h0)
                    slab = bc2p.tile([U, 8, T], dt_e, tag="bc2")
                    src = x0cp_d[h0:h0 + cnt, cols].partition_broadcast(U)
                    nc.sync.dma_start(slab[:, 0:cnt, :], src)
                    bc2.append(slab)

                # ---- layer 1: X1 = W1 @ (X0 (x) X0) + b1 ----
                x1ps = pp1.tile([U, T], F32)
                for k in range(KG1):
                    had1 = hadp.tile([117, T], dt_e, tag="had1")
                    nc.vector.tensor_mul(had1[:], x0st3[:, cols], bc1[:, k, :])
                    nc.tensor.matmul(
                        x1ps[:], w1sb[:, k * U:(k + 1) * U], had1[:],
                        start=(k == 0), stop=(k == KG1 - 1),
                    )
                x1sb = xsbp.tile([U, T], dt_e, tag="x1")
                for s in range(S):
                    bcol = t * S + s
                    nc.scalar.activation(
                        x1sb[:, s * D:(s + 1) * D], x1ps[:, s * D:(s + 1) * D],
                        AF.Identity, bias=b1sb[:], scale=1.0,
                        accum_out=pooled1[:, bcol:bcol + 1],
                    )

                # ---- layer 2: X2 = W2 @ (X0 (x) X1) + b2 ----
                x2ps = pp2.tile([U, T], F32)
                for h in range(F):
                    had2 = hadp.tile([U, T], dt_e, tag="had2")
                    nc.vector.tensor_mul(had2[:], x1sb[:], bc2[h // 8][:, h % 8, :])
                    nc.tensor.matmul(
                        x2ps[:], w2sb[:, h * U:(h + 1) * U], had2[:],
                        start=(h == 0), stop=(h == F - 1),
                    )
                x2sb = xsbp.tile([U, T], dt_e, tag="x2")
                for s in range(S):
                    bcol = t * S + s
                    nc.scalar.activation(
                        x2sb[:, s * D:(s + 1) * D], x2ps[:, s * D:(s + 1) * D],
                        AF.Identity, bias=b2sb[:], scale=1.0,
                        accum_out=pooled2[:, bcol:bcol + 1],
                    )

                # ---- layer 3 Gram: G2[h, b, m] = sum_d X0[h,bd] X2[m,bd] ----
                for s in range(S):
                    b = t * S + s
                    x2t_ps = pptg.tile([D, U], dt_e, tag="tg")
                    nc.tensor.transpose(
                        x2t_ps[:], x2sb[:, s * D:(s + 1) * D], id16[:])
                    x2t = l3p.tile([D, U], dt_e, tag="x2t")
                    nc.scalar.activation(x2t[:], x2t_ps[:], AF.Identity)
                    g2ps = pptg.tile([F, U], F32, tag="tg")
                    nc.tensor.matmul(
                        g2ps[:], x0dt[:, b * F:(b + 1) * F], x2t[:],
                        start=True, stop=True,
                    )
                    nc.scalar.activation(g2sb[:, b, :], g2ps[:], AF.Identity)

            # ---- pooled3 = W3 @ G2 + 64*b3 ----
            with tc.tile_pool(name="ps_tail", bufs=1, space="PSUM") as ppt:
                p3ps = ppt.tile([U, BC], F32, tag="tail")
                for m in range(U):
                    nc.tensor.matmul(
                        p3ps[:], w3sb[:, m * U:(m + 1) * U], g2sb[:, :, m],
                        start=(m == 0), stop=(m == U - 1),
                    )
                nc.scalar.activation(
                    pooled3[:], p3ps[:], AF.Identity, bias=b3sb[:], scale=1.0)

                # ---- transpose pooled_i -> [b, o] and store ----
                for i, pl in enumerate((pooled1, pooled2, pooled3)):
                    trp = ppt.tile([BC, U], F32, tag="tail")
                    nc.tensor.transpose(trp[:], pl[:], id32[:])
                    nc.scalar.activation(
                        outsb[:, i * U:(i + 1) * U], trp[:], AF.Identity)
                nc.sync.dma_start(y_d[:, :], outsb[:])

    nc.compile()
    return nc


def _prep_in_maps(inputs, np_e):
    X0 = np.asarray(inputs["X_0"], np.float32)
    W1 = np.asarray(inputs["W1"], np.float32)
    b1 = np.asarray(inputs["b1"], np.float32)
    W2 = np.asarray(inputs["W2"], np.float32)
    b2 = np.asarray(inputs["b2"], np.float32)
    W3 = np.asarray(inputs["W3"], np.float32)
    b3 = np.asarray(inputs["b3"], np.float32)

    # W1 reorder: [117, 13*128]; rows p=j*39+m, cols k*128+o -> W1[o, (3k+j)*39+m]
    w1r = W1.reshape(U, F, F)                    # [o, h, m]
    w1p = np.zeros((117, KG1 * U), np.float32)
    for k in range(KG1):
        for j in range(3):
            # [m, o] block
            w1p[j * F:(j + 1) * F, k * U:(k + 1) * U] = w1r[:, 3 * k + j, :].T
    w2p = W2.reshape(U, F, U).transpose(2, 1, 0).reshape(U, F * U)   # [m, h*128+o]
    w3p = W3.reshape(U, F, U).transpose(1, 2, 0).reshape(F, U * U)   # [h, m*128+o]

    shared = {
        "w1p": w1p.astype(np_e),
        "w2p": w2p.astype(np_e),
        "w3p": w3p.astype(np_e),
        "b1c": b1.reshape(U, 1).astype(np.float32),
        "b2c": b2.reshape(U, 1).astype(np.float32),
        "b3c": (D * b3).reshape(U, 1).astype(np.float32),
        "id16": np.eye(U, dtype=np_e),
        "id32": np.eye(U, dtype=np.float32),
    }
    in_maps = []
    for c in range(NCORES):
        xs = X0[c * BC:(c + 1) * BC]                         # [128, 39, 64]
        x0cp = xs.transpose(1, 0, 2).reshape(F, BD)          # [h, b*64+d]
        x0dt = xs.transpose(2, 0, 1).reshape(D, BC * F)      # [d, b*39+h]
        m = dict(shared)
        m["x0cp"] = x0cp.astype(np_e)
        m["x0dt"] = x0dt.astype(np_e)
        in_maps.append(m)
    return in_maps


def _run(inputs, trace=False, **kw):
    dt_e, np_e = _dtype_cfg()
    key = dt_e
    if key not in _CACHE:
        _CACHE[key] = _build(dt_e)
    nc = _CACHE[key]
    in_maps = _prep_in_maps(inputs, np_e)
    res = bass_utils.run_bass_kernel_spmd(
        nc, in_maps, core_ids=list(range(NCORES)), trace=trace, **kw)
    y = np.concatenate([r["y"] for r in res.results], axis=0).astype(np.float32)
    return y, res


def kernel(**inputs) -> np.ndarray:
    y, _ = _run(inputs, trace=False)
    return y


# revision 5
# speedup vs baseline: 2.2610x; 2.2610x over previous
"""CIN (xDeepFM Compressed Interaction Network) Bass/Tile kernel for TRN2.

Problem: X_0 [1024, 39, 64]; three CIN layers (units 128 each):
    had_i = outer(X_0, X_i) over channel dims, per (b, d)
    X_{i+1} = W_i @ had_i + b_i            (1x1 conv over channels)
    pooled_i = X_{i+1}.sum(d)
Output: concat(pooled_1..3) -> [1024, 384] fp32.

Strategy (pure data-parallel over batch, 8 cores, 128 samples each):
  * channel-major layout on chip: tensors stored [channels, b*64+d]
  * had formation: DVE tensor_mul against broadcast tiles of X_0 rows,
    materialized by partition-broadcast DMAs straight from compact DRAM
    (0-stride partition APs); 2048-wide super-tiles keep DMA packets at
    4-16KB contiguous runs per partition
  * convs: PE matmuls, k-outer loop accumulating 4 x 512-wide PSUM banks
    per super-tile (layer 1: 13 chunks of 117 = 3 h-values x 39 m;
    layer 2: 39 chunks of 128)
  * evacuation: ScalarE Identity activation with per-partition bias and
    fused accum_out -> pooled_1/pooled_2 sums come free
  * layer 3 never materializes its conv: pooled_3 = W3 @ Gram(X0, X2);
    per 2 samples one PE transpose, then per-sample Gram matmuls with
    output [m, h] so pooled_3 runs as 39 N=128 accumulating matmuls
  * elementwise dtype fp16 by default (DVE 2x mode, ~3e-4 rel err);
    BASS_CIN_DTYPE=fp32 falls back to full fp32.
"""

import os
import numpy as np

import concourse.bass as bass
import concourse.bacc as bacc
import concourse.mybir as mybir
import concourse.tile as tile
from concourse import bass_utils

F32 = mybir.dt.float32
F16 = mybir.dt.float16

B, F, D, U = 1024, 39, 64, 128
NCORES = 8
BC = B // NCORES            # 128 samples per core
BD = BC * D                 # 8192 bd-columns per core
ST = 2048                   # super-tile width (DMA granularity)
NST = BD // ST              # 4
SUB = 512                   # matmul/evac sub-tile width (one PSUM bank)
NSUB = ST // SUB            # 4
SPS = SUB // D              # 8 samples per sub-tile
KG1 = 13                    # layer-1 chunks: 13 x (3 h-values x 39 m) = 117 rows

_CACHE: dict = {}


def _dtype_cfg():
    name = os.environ.get("BASS_CIN_DTYPE", "fp16")
    if name == "fp32":
        return F32, np.float32
    return F16, np.float16


def _build(dt_e) -> bacc.Bacc:
    nc = bacc.Bacc("TRN2", target_bir_lowering=False, debug=False,
                   enable_asserts=False)
    AF = mybir.ActivationFunctionType

    x0cp_d = nc.dram_tensor("x0cp", [F, BD], dt_e, kind="ExternalInput")
    x0dt_d = nc.dram_tensor("x0dt", [D, BC * F], dt_e, kind="ExternalInput")
    w1p_d = nc.dram_tensor("w1p", [117, KG1 * U], dt_e, kind="ExternalInput")
    w2p_d = nc.dram_tensor("w2p", [U, F * U], dt_e, kind="ExternalInput")
    w3p_d = nc.dram_tensor("w3p", [U, F * U], dt_e, kind="ExternalInput")
    b1_d = nc.dram_tensor("b1c", [U, 1], F32, kind="ExternalInput")
    b2_d = nc.dram_tensor("b2c", [U, 1], F32, kind="ExternalInput")
    b3_d = nc.dram_tensor("b3c", [U, 1], F32, kind="ExternalInput")  # 64*b3
    id16_d = nc.dram_tensor("id16", [U, U], dt_e, kind="ExternalInput")
    id32_d = nc.dram_tensor("id32", [U, U], F32, kind="ExternalInput")

    y_d = nc.dram_tensor("y", [BC, 3 * U], F32, kind="ExternalOutput")

    with tile.TileContext(nc) as tc:
        with (
            tc.tile_pool(name="static", bufs=1) as stat,
            tc.tile_pool(name="bc1", bufs=2) as bc1p,
            tc.tile_pool(name="bc2", bufs=3) as bc2p,
            tc.tile_pool(name="had", bufs=3) as hadp,
            tc.tile_pool(name="xsb", bufs=2) as xsbp,
            tc.tile_pool(name="l3sb", bufs=3) as l3p,
            tc.tile_pool(name="ps_conv", bufs=5, space="PSUM") as ppc,
            tc.tile_pool(name="ps_tg", bufs=2, space="PSUM") as pptg,
        ):
            # ---- static loads ----
            x0st3 = stat.tile([117, BD], dt_e)          # X0 rows tiled 3x
            for j in range(3):
                nc.sync.dma_start(x0st3[j * F:(j + 1) * F, :], x0cp_d[:, :])
            w1sb = stat.tile([117, KG1 * U], dt_e)
            nc.sync.dma_start(w1sb[:], w1p_d[:, :])
            w2sb = stat.tile([U, F * U], dt_e)
            nc.sync.dma_start(w2sb[:], w2p_d[:, :])
            w3sb = stat.tile([U, F * U], dt_e)
            nc.sync.dma_start(w3sb[:], w3p_d[:, :])
            # two stacked copies (partitions 0-63 and 64-127) so per-sample
            # Gram matmuls can match lhsT base_partition for both halves
            x0dt = stat.tile([2 * D, BC * F], dt_e)
            nc.sync.dma_start(x0dt[0:D, :], x0dt_d[:, :])
            nc.sync.dma_start(x0dt[D:2 * D, :], x0dt_d[:, :])
            b1sb = stat.tile([U, 1], F32)
            nc.sync.dma_start(b1sb[:], b1_d[:, :])
            b2sb = stat.tile([U, 1], F32)
            nc.sync.dma_start(b2sb[:], b2_d[:, :])
            b3sb = stat.tile([U, 1], F32)
            nc.sync.dma_start(b3sb[:], b3_d[:, :])
            id16 = stat.tile([U, U], dt_e)
            nc.sync.dma_start(id16[:], id16_d[:, :])
            id32 = stat.tile([U, U], F32)
            nc.sync.dma_start(id32[:], id32_d[:, :])

            pooled1 = stat.tile([U, BC], F32)
            pooled2 = stat.tile([U, BC], F32)
            pooled3 = stat.tile([U, BC], F32)
            g2f = stat.tile([U, F, BC], dt_e)           # Gram: [m, h, b]
            outsb = stat.tile([BC, 3 * U], F32)

            # ---- main loop over super-tiles ----
            for st in range(NST):
                cols = slice(st * ST, (st + 1) * ST)

                # broadcast slabs for this super-tile (partition-broadcast DMA)
                bc1g = []
                for kg in range((KG1 + 3) // 4):         # k-groups of 4 (last 1)
                    k0 = kg * 4
                    kcnt = min(4, KG1 - k0)
                    slab = bc1p.tile([117, 4, ST], dt_e, tag="bc1")
                    for j in range(3):
                        src = x0cp_d[3 * k0 + j:3 * (k0 + kcnt):3, cols] \
                            .partition_broadcast(F)
                        nc.sync.dma_start(slab[j * F:(j + 1) * F, 0:kcnt, :], src)
                    bc1g.append(slab)
                bc2g = []
                for hg in range((F + 3) // 4):           # h-groups of 4 (last 3)
                    h0 = hg * 4
                    hcnt = min(4, F - h0)
                    slab = bc2p.tile([U, 4, ST], dt_e, tag="bc2")
                    src = x0cp_d[h0:h0 + hcnt, cols].partition_broadcast(U)
                    nc.sync.dma_start(slab[:, 0:hcnt, :], src)
                    bc2g.append(slab)

                # ---- layer 1: X1 = W1 @ (X0 (x) X0) + b1 ----
                x1ps = [ppc.tile([U, SUB], F32, tag="conv", name=f"x1ps{i}")
                         for i in range(NSUB)]
                for k in range(KG1):
                    had1 = hadp.tile([117, ST], dt_e, tag="had1")
                    nc.vector.tensor_mul(
                        had1[:], x0st3[:, cols], bc1g[k // 4][:, k % 4, :])
                    for sb_i in range(NSUB):
                        nc.tensor.matmul(
                            x1ps[sb_i][:], w1sb[:, k * U:(k + 1) * U],
                            had1[:, sb_i * SUB:(sb_i + 1) * SUB],
                            start=(k == 0), stop=(k == KG1 - 1),
                        )
                x1sb = xsbp.tile([U, ST], dt_e, tag="x1")
                for sb_i in range(NSUB):
                    for s in range(SPS):
                        bcol = st * (ST // D) + sb_i * SPS + s
                        o0 = sb_i * SUB + s * D
                        nc.scalar.activation(
                            x1sb[:, o0:o0 + D], x1ps[sb_i][:, s * D:(s + 1) * D],
                            AF.Identity, bias=b1sb[:], scale=1.0,
                            accum_out=pooled1[:, bcol:bcol + 1],
                        )

                # ---- layer 2: X2 = W2 @ (X0 (x) X1) + b2 ----
                x2ps = [ppc.tile([U, SUB], F32, tag="conv", name=f"x2ps{i}")
                         for i in range(NSUB)]
                for h in range(F):
                    had2 = hadp.tile([U, ST], dt_e, tag="had2")
                    nc.vector.tensor_mul(
                        had2[:], x1sb[:], bc2g[h // 4][:, h % 4, :])
                    for sb_i in range(NSUB):
                        nc.tensor.matmul(
                            x2ps[sb_i][:], w2sb[:, h * U:(h + 1) * U],
                            had2[:, sb_i * SUB:(sb_i + 1) * SUB],
                            start=(h == 0), stop=(h == F - 1),
                        )
                x2sb = xsbp.tile([U, ST], dt_e, tag="x2")
                for sb_i in range(NSUB):
                    for s in range(SPS):
                        bcol = st * (ST // D) + sb_i * SPS + s
                        o0 = sb_i * SUB + s * D
                        nc.scalar.activation(
                            x2sb[:, o0:o0 + D], x2ps[sb_i][:, s * D:(s + 1) * D],
                            AF.Identity, bias=b2sb[:], scale=1.0,
                            accum_out=pooled2[:, bcol:bcol + 1],
                        )

                # ---- layer 3 Gram: G2[m, h, b] = sum_d X2[m,bd] X0[h,bd] ----
                for s2 in range(ST // D // 2):           # 2 samples per transpose
                    x2t_ps = pptg.tile([U, U], dt_e, tag="tg")
                    nc.tensor.transpose(
                        x2t_ps[:], x2sb[:, s2 * 2 * D:(s2 + 1) * 2 * D], id16[:])
                    x2t = l3p.tile([U, U], dt_e, tag="x2t")
                    nc.scalar.activation(x2t[:], x2t_ps[:], AF.Identity)
                    for ls in range(2):
                        b = st * (ST // D) + s2 * 2 + ls
                        g2ps = pptg.tile([U, F], F32, tag="tg")
                        nc.tensor.matmul(
                            g2ps[:], x2t[ls * D:(ls + 1) * D, :],
                            x0dt[ls * D:(ls + 1) * D, b * F:(b + 1) * F],
                            start=True, stop=True,
                        )
                        nc.scalar.activation(g2f[:, :, b], g2ps[:], AF.Identity)

            # ---- pooled3 = W3 @ G2 + 64*b3 ----
            with tc.tile_pool(name="ps_tail", bufs=1, space="PSUM") as ppt:
                p3ps = ppt.tile([U, BC], F32, tag="tail")
                for h in range(F):
                    nc.tensor.matmul(
                        p3ps[:], w3sb[:, h * U:(h + 1) * U], g2f[:, h, :],
                        start=(h == 0), stop=(h == F - 1),
                    )
                nc.scalar.activation(
                    pooled3[:], p3ps[:], AF.Identity, bias=b3sb[:], scale=1.0)

                # ---- transpose pooled_i -> [b, o] and store ----
                for i, pl in enumerate((pooled1, pooled2, pooled3)):
                    trp = ppt.tile([BC, U], F32, tag="tail")
                    nc.tensor.transpose(trp[:], pl[:], id32[:])
                    nc.scalar.activation(
                        outsb[:, i * U:(i + 1) * U], trp[:], AF.Identity)
                nc.sync.dma_start(y_d[:, :], outsb[:])

    nc.compile()
    return nc


def _prep_in_maps(inputs, np_e):
    X0 = np.asarray(inputs["X_0"], np.float32)
    W1 = np.asarray(inputs["W1"], np.float32)
    b1 = np.asarray(inputs["b1"], np.float32)
    W2 = np.asarray(inputs["W2"], np.float32)
    b2 = np.asarray(inputs["b2"], np.float32)
    W3 = np.asarray(inputs["W3"], np.float32)
    b3 = np.asarray(inputs["b3"], np.float32)

    # W1 reorder: [117, 13*128]; rows p=j*39+m, cols k*128+o -> W1[o, (3k+j)*39+m]
    w1r = W1.reshape(U, F, F)                    # [o, h, m]
    w1p = np.zeros((117, KG1 * U), np.float32)
    for k in range(KG1):
        for j in range(3):
            w1p[j * F:(j + 1) * F, k * U:(k + 1) * U] = w1r[:, 3 * k + j, :].T
    # [m, h*128+o]
    w2p = W2.reshape(U, F, U).transpose(2, 1, 0).reshape(U, F * U)
    w3p = W3.reshape(U, F, U).transpose(2, 1, 0).reshape(U, F * U)

    shared = {
        "w1p": w1p.astype(np_e),
        "w2p": w2p.astype(np_e),
        "w3p": w3p.astype(np_e),
        "b1c": b1.reshape(U, 1).astype(np.float32),
        "b2c": b2.reshape(U, 1).astype(np.float32),
        "b3c": (D * b3).reshape(U, 1).astype(np.float32),
        "id16": np.eye(U, dtype=np_e),
        "id32": np.eye(U, dtype=np.float32),
    }
    in_maps = []
    for c in range(NCORES):
        xs = X0[c * BC:(c + 1) * BC]                         # [128, 39, 64]
        x0cp = xs.transpose(1, 0, 2).reshape(F, BD)          # [h, b*64+d]
        x0dt = xs.transpose(2, 0, 1).reshape(D, BC * F)      # [d, b*39+h]
        m = dict(shared)
        m["x0cp"] = x0cp.astype(np_e)
        m["x0dt"] = x0dt.astype(np_e)
        in_maps.append(m)
    return in_maps


def _run(inputs, trace=False, **kw):
    dt_e, np_e = _dtype_cfg()
    key = dt_e
    if key not in _CACHE:
        _CACHE[key] = _build(dt_e)
    nc = _CACHE[key]
    in_maps = _prep_in_maps(inputs, np_e)
    res = bass_utils.run_bass_kernel_spmd(
        nc, in_maps, core_ids=list(range(NCORES)), trace=trace, **kw)
    y = np.concatenate([r["y"] for r in res.results], axis=0).astype(np.float32)
    return y, res


def kernel(**inputs) -> np.ndarray:
    y, _ = _run(inputs, trace=False)
    return y
